# revision 16
# baseline (speedup 1.0000x reference)
"""GEAR quantized-KV Llama attention decode step on 8 trn2 NeuronCores.

Sharding: tensor-parallel over heads (4 heads/core x 8 cores), all batches on
every core; each core computes a partial wo-product, summed on host.

v3: fp8 codes + fp8 matmuls (DoubleRow on V side), merged per-pair blob DMAs,
software-pipelined K/V phases, early wo load, DVE-staged copies.
"""
import os
import sys
import math

sys.path.insert(0, "/opt/trn_rl_repo")
import numpy as np
from contextlib import ExitStack

import concourse.bass as bass
import concourse.mybir as mybir
import concourse.tile as tile
from concourse import bacc, bass_isa
from concourse.bass_utils import run_bass_kernel_spmd
from concourse.masks import make_identity

B, H, D, HID = 4, 32, 128, 4096
SQ, SF, QL = 4096, 63, 1
GS, RANK = 64, 4
THETA = 10000.0
NCORES = 8
HPC = H // NCORES          # heads per core = 4
NP = B * HPC               # (b,h) pairs per core = 16
NCH = SQ // 128            # 32 s-chunks
G = SQ // GS               # 64 groups along seq (K side)
FD = D // GS               # 2 groups along head_dim (V side)
SFP = SF + 1               # 64 full-precision keys incl the new token
DT = mybir.dt
ISQD = 1.0 / math.sqrt(D)
WS = 16.0                  # fp8 weight pre-scale
QS = 256.0                 # fp8 qs pre-scale (folded into kscale on host)
VS = 16.0                  # fp8 aw*vscale pre-scale
VTW = 160                  # vt row: 128 codes | e | pad | aw3ext@144 (16B-aligned)
BLOBW = 64 + 68 + NCH * RANK + NCH * FD  # 324

_CACHE = {}


def _build():
    nc = bacc.Bacc("TRN2", target_bir_lowering=False)
    f32, bf16, fp8 = DT.float32, DT.bfloat16, DT.float8e4

    # ---- DRAM parameters (per core) ----
    hid8 = nc.declare_dram_parameter("hid8", [128, 32, B], fp8, isOutput=False)
    w8 = {w: nc.declare_dram_parameter(w, [128, 32, HPC * D], fp8, isOutput=False)
          for w in ("wq8", "wk8", "wv8")}
    woT = nc.declare_dram_parameter("woT", [128, HPC, HID], bf16, isOutput=False)
    ropeM = nc.declare_dram_parameter("ropeM", [128, B, 128], bf16, isOutput=False)
    blob = nc.declare_dram_parameter("blob", [128, NP, BLOBW], bf16, isOutput=False)
    vpmn = nc.declare_dram_parameter("vpmn", [8, NP, 128], bf16, isOutput=False)
    kfull = nc.declare_dram_parameter("kfull", [128, NP, SFP], bf16, isOutput=False)
    vfull = nc.declare_dram_parameter("vfull", [SFP, NP, 128], bf16, isOutput=False)
    kc8 = nc.declare_dram_parameter("kc8", [NP // 2, 128, 2 * SQ], fp8, isOutput=False)
    vt8 = nc.declare_dram_parameter("vt8", [NP, 128, NCH, VTW], fp8, isOutput=False)
    out = nc.declare_dram_parameter("out", [B, HID], f32, isOutput=True)

    AO = mybir.AluOpType
    AF = mybir.ActivationFunctionType
    PM = mybir.MatmulPerfMode

    with tile.TileContext(nc) as tc, ExitStack() as ctx:
        const = ctx.enter_context(tc.tile_pool(name="const", bufs=1))
        pw = ctx.enter_context(tc.tile_pool(name="pw", bufs=1))
        pctx = ctx.enter_context(ExitStack())
        psP = pctx.enter_context(tc.tile_pool(name="psP", bufs=1, space="PSUM"))
        psR = pctx.enter_context(tc.tile_pool(name="psR", bufs=2, space="PSUM"))

        pkc = ctx.enter_context(tc.tile_pool(name="pkc", bufs=2))
        pvt = ctx.enter_context(tc.tile_pool(name="pvt", bufs=4))

        # ---- constants ----
        id4 = const.tile([4, 4], f32)
        make_identity(nc, id4[:])

        # ---- upfront loads, interleaved with first code DMAs ----
        hid_sb = const.tile([128, 32, B], fp8)
        nc.sync.dma_start(out=hid_sb[:], in_=hid8[:])
        ropeM_sb = const.tile([128, B, 128], bf16)
        nc.sync.dma_start(out=ropeM_sb[:], in_=ropeM[:])
        blob_sb = const.tile([128, NP, BLOBW], bf16)
        nc.scalar.dma_start(out=blob_sb[:], in_=blob[:])
        wslabs = {}
        kcts, vtts = [], []
        for i, wname in enumerate(("wq8", "wk8", "wv8")):
            slab = pw.tile([128, 32, HPC * D], fp8, tag="slab" + wname)
            nc.sync.dma_start(out=slab[:], in_=w8[wname][:])
            wslabs[wname] = slab
            if i < 2:
                kct = pkc.tile([128, 2 * SQ], fp8, tag="kc")
                nc.sync.dma_start(out=kct[:], in_=kc8[i])
                kcts.append(kct)
            vtt = pvt.tile([128, NCH, VTW], fp8, tag="vt")
            nc.scalar.dma_start(out=vtt[:], in_=vt8[i])
            vtts.append(vtt)
        vpmn_sb = const.tile([8, NP, 128], bf16)
        nc.scalar.dma_start(out=vpmn_sb[:], in_=vpmn[:])
        kfull_sb = const.tile([128, NP, SFP], bf16)
        nc.scalar.dma_start(out=kfull_sb[:], in_=kfull[:])
        vfull_sb = const.tile([SFP, NP, 128], bf16)
        nc.scalar.dma_start(out=vfull_sb[:], in_=vfull[:])
        wo_sb = const.tile([128, HPC, HID], bf16)
        nc.scalar.dma_start(out=wo_sb[:], in_=woT[:])

        # ---- projections (plain fp8): pps[b, 512] = hid @ w.T * WS ----
        proj_ps = {}
        for wname in ("wq8", "wk8", "wv8"):
            pps = psP.tile([B, HPC * D], f32, tag="proj" + wname)
            for kk in range(32):
                nc.tensor.matmul(pps[:], hid_sb[:, kk, :], wslabs[wname][:, kk, :],
                                 start=(kk == 0), stop=(kk == 31))
            proj_ps[wname] = pps

        # v in row layout (for the new-token value row)
        v_sb = const.tile([B, HPC * D], bf16)
        nc.scalar.mul(v_sb[:], proj_ps["wv8"][:], 1.0 / WS)

        # ---- q/k: psum rows -> sbuf -> per-head transpose -> RoPE matmul ----
        qscT = const.tile([128, NP], bf16)   # cols idx = h*B+b, scaled 1/sqrt(D)
        kT = const.tile([128, NP], bf16)
        for wname, dst, scale in (("wq8", qscT, ISQD / WS), ("wk8", kT, 1.0 / WS)):
            row_sb = const.tile([B, HPC * D], f32, tag="row" + wname)
            nc.scalar.copy(row_sb[:], proj_ps[wname][:])
            colT = const.tile([128, HPC, B], bf16, tag="colT" + wname)
            for h in range(HPC):
                pt = psR.tile([128, B], f32, tag="tmp")
                nc.tensor.transpose(pt[:], row_sb[0:B, h * D:(h + 1) * D], id4[:])
                nc.scalar.copy(colT[:, h, :], pt[:])
            dstv = dst[:].rearrange("p (h b) -> p h b", b=B)
            for b in range(B):
                ro = psR.tile([128, HPC], f32, tag="tmp")
                nc.tensor.matmul(ro[:], ropeM_sb[:, b, :], colT[:, :, b],
                                 start=True, stop=True)
                nc.scalar.mul(dstv[:, :, b], ro[:], scale)

        # new-token k/v into resident full-precision tiles (one DMA each)
        nc.sync.dma_start(out=kfull_sb[:, :, SF:SFP], in_=kT[:])
        nc.scalar.dma_start(out=vfull_sb[SF:SFP, :, :], in_=v_sb[:])

        pctx.close()
        ictx = ctx.enter_context(ExitStack())
        psml = ictx.enter_context(tc.tile_pool(name="psml", bufs=3))
        psK = ictx.enter_context(tc.tile_pool(name="psK", bufs=2, space="PSUM"))
        psV = ictx.enter_context(tc.tile_pool(name="psV", bufs=2, space="PSUM"))
        psM = ictx.enter_context(tc.tile_pool(name="psM", bufs=3, space="PSUM"))

        woin_sb = const.tile([128, NP], bf16)
        state = {}

        def emit_K(idx):
            qcol = qscT[:, idx:idx + 1]
            if idx % 2 == 0 and idx >= 4:
                kct = pkc.tile([128, 2 * SQ], fp8, tag="kc")
                nc.sync.dma_start(out=kct[:], in_=kc8[idx // 2])
                kcts.append(kct)
            kct = kcts[idx // 2]
            if idx < 3:
                vtt = vtts[idx]
            else:
                vtt = pvt.tile([128, NCH, VTW], fp8, tag="vt")
                nc.scalar.dma_start(out=vtt[:], in_=vt8[idx])
            kcv = kct[:].rearrange("p (two s) -> p two s", two=2)[:, idx % 2, :]

            # qs8[d, g] = q_d * kscale*256 (fp8)
            qs8 = psml.tile([128, G], fp8, tag="qs8")
            nc.vector.tensor_tensor(qs8[:], blob_sb[:, idx, 0:64],
                                    qcol.to_broadcast((128, G)), AO.mult)
            psk = psK.tile([128, 2 * NCH], f32, tag="psk")
            for c in range(NCH):
                nc.tensor.matmul(psk[:, 2 * c:2 * c + 2], kcv[:, c * 128:(c + 1) * 128],
                                 qs8[:, 2 * c:2 * c + 2], start=True, stop=True)
            psm = psM.tile([128, 80], f32, tag="psm")
            nc.tensor.matmul(psm[0:SFP, 0:1], kfull_sb[:, idx, :], qcol,
                             start=True, stop=True)
            nc.tensor.matmul(psm[0:1, 4:72], qcol, blob_sb[:, idx, 64:132],
                             start=True, stop=True)
            qrmn_sb = psml.tile([1, 68], bf16, tag="qrmn")
            nc.vector.tensor_copy(qrmn_sb[:], psm[0:1, 4:72])
            qrb = psml.tile([128, 68], bf16, tag="qrb")
            nc.gpsimd.partition_broadcast(qrb[:], qrmn_sb[:])

            kpv = blob_sb[:, idx, 132:132 + NCH * RANK].rearrange(
                "p (c r) -> p c r", r=RANK)
            lrt = psml.tile([128, NCH, RANK], f32, tag="lrt")
            nc.vector.tensor_tensor(lrt[:], kpv,
                                    qrb[:, None, 0:4].to_broadcast((128, NCH, RANK)),
                                    AO.mult)
            lr = psml.tile([128, NCH], f32, tag="lr")
            nc.vector.reduce_sum(lr[:], lrt[:], axis=mybir.AxisListType.X)

            att = psml.tile([128, NCH + 1], f32, tag="att")
            pskv = psk[:].rearrange("p (c two) -> p c two", two=2)
            bbv = qrb[:, 4:68].rearrange("p (c two) -> p c two", two=2)
            nc.vector.scalar_tensor_tensor(att[0:64, 0:NCH], pskv[0:64, :, 0],
                                           1.0 / QS, lr[0:64, :], AO.mult, AO.add)
            nc.vector.scalar_tensor_tensor(att[64:128, 0:NCH], pskv[64:128, :, 1],
                                           1.0 / QS, lr[64:128, :], AO.mult, AO.add)
            nc.vector.tensor_tensor(att[0:64, 0:NCH], att[0:64, 0:NCH],
                                    bbv[0:64, :, 0], AO.add)
            nc.vector.tensor_tensor(att[64:128, 0:NCH], att[64:128, 0:NCH],
                                    bbv[64:128, :, 1], AO.add)
            nc.vector.tensor_copy(att[0:SFP, NCH:NCH + 1], psm[0:SFP, 0:1])
            nc.vector.memset(att[SFP:128, NCH:NCH + 1], -1e30)

            e = psml.tile([128, NCH + 1], bf16, tag="e")
            ssum = psml.tile([128, 1], f32, tag="ssum")
            nc.scalar.activation(e[:], att[:], AF.Exp, accum_out=ssum[:])
            sg = psml.tile([128, 1], f32, tag="sg")
            nc.gpsimd.partition_all_reduce(sg[:], ssum[:], 128, bass_isa.ReduceOp.add)
            state[idx] = (vtt, psm, e, sg)

        def emit_V(idx):
            vtt, psm, e, sg = state.pop(idx)
            vidx = (idx % B) * HPC + idx // B
            recip = psml.tile([128, 1], f32, tag="recip")
            nc.vector.reciprocal(recip[:], sg[:])
            recipV = psml.tile([128, 1], f32, tag="recipV")
            nc.vector.tensor_scalar(recipV[:], recip[:], 1.0 / VS, None, AO.mult)

            vscv = blob_sb[:, idx, 260:324].rearrange("p (c two) -> p c two", two=2)
            nc.vector.scalar_tensor_tensor(
                vtt[:, :, 144:146], e[:, 0:NCH, None].to_broadcast((128, NCH, 2)),
                VS, vscv, AO.mult, AO.mult)
            nc.vector.tensor_copy(vtt[:, :, 128:129], e[:, 0:NCH, None])

            psv = psV.tile([16, 132], f32, tag="psv")
            for c in range(NCH // 2):
                nc.tensor.matmul(psv[:], vtt[:, 2 * c:2 * c + 2, 144:160],
                                 vtt[:, 2 * c:2 * c + 2, 0:132],
                                 start=(c == 0), stop=(c == NCH // 2 - 1),
                                 perf_mode=PM.DoubleRow)

            awf = psml.tile([SFP, 1], bf16, tag="awf")
            nc.vector.tensor_scalar(awf[:], e[0:SFP, NCH:NCH + 1],
                                    recip[0:SFP, 0:1], None, AO.mult)
            nc.tensor.matmul(psm[:, 76:77], vfull_sb[:, vidx, :], awf[:],
                             start=True, stop=False)
            colsb = psml.tile([8, 1], bf16, tag="colsb")
            nc.vector.tensor_scalar(colsb[:], psv[0:8, 128:129],
                                    recip[0:8, 0:1], None, AO.mult)
            nc.tensor.matmul(psm[:, 76:77], vpmn_sb[:, idx, :], colsb[:],
                             start=False, stop=True)

            vT = psml.tile([2, 128], f32, tag="vT")
            nc.vector.tensor_copy(vT[:], psv[0:2, 0:128])
            nc.tensor.transpose(psm[:, 74:76], vT[:], id4[0:2, 0:2])
            wsb = psml.tile([128, 1], f32, tag="wsb")
            nc.vector.tensor_copy(wsb[:], psm[:, 76:77])
            nc.vector.scalar_tensor_tensor(woin_sb[0:64, idx:idx + 1],
                                           psm[0:64, 74:75], recipV[0:64, 0:1],
                                           wsb[0:64, :], AO.mult, AO.add)
            nc.vector.scalar_tensor_tensor(woin_sb[64:128, idx:idx + 1],
                                           psm[64:128, 75:76], recipV[64:128, 0:1],
                                           wsb[64:128, :], AO.mult, AO.add)

        # software pipeline: K(i) ahead of V(i-1)
        for i in range(NP + 1):
            if i < NP:
                emit_K(i)
            if i > 0:
                emit_V(i - 1)

        # ---- tail: wo matmul ----
        ictx.close()
        psO = ctx.enter_context(tc.tile_pool(name="psO", bufs=2, space="PSUM"))
        for half in range(2):
            po = psO.tile([B, HID // 2], f32, tag="po")
            for h in range(HPC):
                for nb in range(4):
                    j0 = half * 2048 + nb * 512
                    nc.tensor.matmul(po[:, nb * 512:(nb + 1) * 512],
                                     woin_sb[:, h * B:(h + 1) * B],
                                     wo_sb[:, h, j0:j0 + 512],
                                     start=(h == 0), stop=(h == HPC - 1))
            osb = const.tile([B, HID // 2], f32, tag=f"osb{half}")
            nc.scalar.copy(osb[:], po[:])
            nc.sync.dma_start(out=out[:, half * 2048:(half + 1) * 2048], in_=osb[:])

    nc.compile()
    return nc


def _host_prep(inputs):
    f8 = mybir.dt.np(mybir.dt.float8e4)
    bf = mybir.dt.np(mybir.dt.bfloat16)
    hs = np.asarray(inputs["hidden_states"], np.float32)
    pos = np.asarray(inputs["position_ids"])

    # rope matrices M_b^T (bf16): q_roped = M_b @ q
    inv = 1.0 / (THETA ** (np.arange(0, D, 2, dtype=np.float32) / D))
    fr = pos[:, 0].astype(np.float32)[:, None] * inv[None, :]
    emb = np.concatenate([fr, fr], axis=1)          # [B, 128]
    cos_b, sin_b = np.cos(emb), np.sin(emb)
    M = np.zeros((B, D, D), np.float32)
    dd = np.arange(D)
    M[:, dd, dd] = cos_b
    M[:, dd[:64], dd[:64] + 64] = -sin_b[:, :64]
    M[:, dd[64:], dd[64:] - 64] = sin_b[:, 64:]
    ropeM = np.ascontiguousarray(
        M.transpose(2, 0, 1)).astype(bf)             # [128(k), B, 128(m)]

    hidT = hs[:, 0, :].T                             # [HID, B]
    hid8 = np.ascontiguousarray(
        hidT.reshape(32, 128, B).transpose(1, 0, 2)).astype(f8)

    wq, wk, wv, wo = (np.asarray(inputs[k], np.float32) for k in ("wq", "wk", "wv", "wo"))
    kq_all = np.asarray(inputs["k_quant"], np.int32)
    ks_all = np.asarray(inputs["k_scale"], np.float32)
    km_all = np.asarray(inputs["k_mn"], np.float32)
    kf_all = np.asarray(inputs["k_full"], np.float32)
    kp_all = np.asarray(inputs["key_p"], np.float32)
    keyq_all = np.asarray(inputs["key_q"], np.float32)
    vq_all = np.asarray(inputs["v_quant"], np.int32)
    vs_all = np.asarray(inputs["v_scale"], np.float32)
    vm_all = np.asarray(inputs["v_mn"], np.float32)
    vf_all = np.asarray(inputs["v_full"], np.float32)
    vvq_all = np.asarray(inputs["value_q"], np.float32)
    vvp_all = np.asarray(inputs["value_p"], np.float32)

    in_maps = []
    for core in range(NCORES):
        h0 = core * HPC
        sl = slice(h0 * D, (h0 + HPC) * D)
        hsl = slice(h0, h0 + HPC)

        def wslab(w):   # [512, HID] -> [128, 32, 512] fp8 (x WS)
            wT = w[sl].T * WS
            return np.ascontiguousarray(
                wT.reshape(32, 128, HPC * D).transpose(1, 0, 2)).astype(f8)

        blobc = np.zeros((128, NP, BLOBW), np.float32)
        kfullc = np.zeros((128, NP, SFP), np.float32)
        vfullc = np.zeros((SFP, NP, 128), np.float32)
        vpmnc = np.zeros((8, NP, 128), np.float32)
        for h in range(HPC):
            for b in range(B):
                idx = h * B + b
                vidx = b * HPC + h
                blobc[:, idx, 0:64] = ks_all[b, h0 + h] * QS
                blobc[:, idx, 64:68] = keyq_all[b, h0 + h]
                blobc[:, idx, 68:132] = km_all[b, h0 + h]
                kp = kp_all[b, h0 + h].reshape(NCH, 128, RANK)
                blobc[:, idx, 132:132 + NCH * RANK] = \
                    kp.transpose(1, 0, 2).reshape(128, NCH * RANK)
                vs = vs_all[b, h0 + h].reshape(NCH, 128, FD)
                blobc[:, idx, 260:324] = vs.transpose(1, 0, 2).reshape(128, NCH * FD)
                kfullc[:, idx, 0:SF] = kf_all[b, h0 + h].T
                vfullc[0:SF, vidx, :] = vf_all[b, h0 + h]
                vpmnc[2:6, idx, :] = vvp_all[b, h0 + h].T
                vpmnc[6, idx, 0:64] = 1.0
                vpmnc[7, idx, 64:128] = 1.0

        # kc8 [NP//2, 128, 2*SQ]: pairs (2j, 2j+1) share one DMA
        kq = kq_all[:, hsl]                          # [B, HPC, 128, SQ]
        kcc = np.empty((NP // 2, 128, 2 * SQ), f8)
        for j in range(NP // 2):
            for t in range(2):
                idx = 2 * j + t
                h, b = idx // B, idx % B
                kcc[j, :, t * SQ:(t + 1) * SQ] = kq[b, h].astype(f8)

        # vt blob [NP, 128, NCH, VTW]: codes | e-slot | pad | aw3ext(16)
        vq = vq_all[:, hsl].reshape(B, HPC, NCH, 128, D)
        vvq = vvq_all[:, hsl].reshape(B, HPC, NCH, 128, RANK)
        vm = vm_all[:, hsl].reshape(B, HPC, NCH, 128, FD)
        vtc = np.zeros((NP, 128, NCH, VTW), f8)
        for h in range(HPC):
            for b in range(B):
                idx = h * B + b
                vtc[idx, :, :, 0:128] = vq[b, h].transpose(1, 0, 2).astype(f8)
                vtc[idx, :, :, 146:150] = vvq[b, h].transpose(1, 0, 2).astype(f8)
                vtc[idx, :, :, 150:152] = vm[b, h].transpose(1, 0, 2).astype(f8)

        m = {
            "hid8": hid8, "ropeM": ropeM,
            "wq8": wslab(wq), "wk8": wslab(wk), "wv8": wslab(wv),
            "woT": np.ascontiguousarray(
                wo[:, sl].T.reshape(HPC, 128, HID).transpose(1, 0, 2)).astype(bf),
            "blob": blobc.astype(bf),
            "vpmn": vpmnc.astype(bf),
            "kfull": kfullc.astype(bf),
            "vfull": vfullc.astype(bf),
            "kc8": kcc,
            "vt8": vtc,
        }
        in_maps.append(m)
    return in_maps


def kernel(**inputs):
    if "nc" not in _CACHE:
        _CACHE["nc"] = _build()
    nc = _CACHE["nc"]
    in_maps = _host_prep(inputs)
    res = run_bass_kernel_spmd(nc, in_maps, list(range(NCORES)),
                               trace=bool(os.environ.get("K_TRACE")))
    kernel.last = res
    total = np.zeros((B, HID), np.float32)
    for r in res.results:
        total += r["out"]
    return total.reshape(B, QL, HID)


# revision 17
# speedup vs baseline: 1.0182x; 1.0182x over previous
"""GEAR quantized-KV Llama attention decode step on 8 trn2 NeuronCores.

Sharding: tensor-parallel over heads (4 heads/core x 8 cores), all batches on
every core; each core computes a partial wo-product, summed on host.

v3: fp8 codes + fp8 matmuls (DoubleRow on V side), merged per-pair blob DMAs,
software-pipelined K/V phases, early wo load, DVE-staged copies.
"""
import os
import sys
import math

sys.path.insert(0, "/opt/trn_rl_repo")
import numpy as np
from contextlib import ExitStack

import concourse.bass as bass
import concourse.mybir as mybir
import concourse.tile as tile
from concourse import bacc, bass_isa
from concourse.bass_utils import run_bass_kernel_spmd
from concourse.masks import make_identity

B, H, D, HID = 4, 32, 128, 4096
SQ, SF, QL = 4096, 63, 1
GS, RANK = 64, 4
THETA = 10000.0
NCORES = 8
HPC = H // NCORES          # heads per core = 4
NP = B * HPC               # (b,h) pairs per core = 16
NCH = SQ // 128            # 32 s-chunks
G = SQ // GS               # 64 groups along seq (K side)
FD = D // GS               # 2 groups along head_dim (V side)
SFP = SF + 1               # 64 full-precision keys incl the new token
DT = mybir.dt
ISQD = 1.0 / math.sqrt(D)
WS = 16.0                  # fp8 weight pre-scale
QS = 256.0                 # fp8 qs pre-scale (folded into kscale on host)
VS = 16.0                  # fp8 aw*vscale pre-scale
VTW = 160                  # vt row: 128 codes | e | pad | aw3ext@144 (16B-aligned)
BLOBW = 64 + 68 + NCH * RANK + NCH * FD  # 324

_CACHE = {}


def _build():
    nc = bacc.Bacc("TRN2", target_bir_lowering=False)
    f32, bf16, fp8 = DT.float32, DT.bfloat16, DT.float8e4

    # ---- DRAM parameters (per core) ----
    hid8 = nc.declare_dram_parameter("hid8", [128, 32, B], fp8, isOutput=False)
    w8 = {w: nc.declare_dram_parameter(w, [128, 32, HPC * D], fp8, isOutput=False)
          for w in ("wq8", "wk8", "wv8")}
    woT = nc.declare_dram_parameter("woT", [128, HPC, HID], bf16, isOutput=False)
    ropeM = nc.declare_dram_parameter("ropeM", [128, B, 128], bf16, isOutput=False)
    blob = nc.declare_dram_parameter("blob", [128, NP, BLOBW], bf16, isOutput=False)
    vpmn = nc.declare_dram_parameter("vpmn", [8, NP, 128], bf16, isOutput=False)
    kfull = nc.declare_dram_parameter("kfull", [128, NP, SFP], bf16, isOutput=False)
    vfull = nc.declare_dram_parameter("vfull", [SFP, NP, 128], bf16, isOutput=False)
    kc8 = nc.declare_dram_parameter("kc8", [NP // 2, 128, 2 * SQ], fp8, isOutput=False)
    vt8 = nc.declare_dram_parameter("vt8", [NP, 128, NCH, VTW], fp8, isOutput=False)
    out = nc.declare_dram_parameter("out", [B, HID], f32, isOutput=True)

    AO = mybir.AluOpType
    AF = mybir.ActivationFunctionType
    PM = mybir.MatmulPerfMode

    with tile.TileContext(nc) as tc, ExitStack() as ctx:
        const = ctx.enter_context(tc.tile_pool(name="const", bufs=1))
        pw = ctx.enter_context(tc.tile_pool(name="pw", bufs=1))
        pctx = ctx.enter_context(ExitStack())
        psP = pctx.enter_context(tc.tile_pool(name="psP", bufs=1, space="PSUM"))
        psR = pctx.enter_context(tc.tile_pool(name="psR", bufs=2, space="PSUM"))

        pkc = ctx.enter_context(tc.tile_pool(name="pkc", bufs=2))
        pvt = ctx.enter_context(tc.tile_pool(name="pvt", bufs=4))

        # ---- constants ----
        id4 = const.tile([4, 4], f32)
        make_identity(nc, id4[:])

        # ---- upfront loads, interleaved with first code DMAs ----
        hid_sb = const.tile([128, 32, B], fp8)
        nc.sync.dma_start(out=hid_sb[:], in_=hid8[:])
        ropeM_sb = const.tile([128, B, 128], bf16)
        nc.sync.dma_start(out=ropeM_sb[:], in_=ropeM[:])
        blob_sb = const.tile([128, NP, BLOBW], bf16)
        nc.scalar.dma_start(out=blob_sb[:], in_=blob[:])
        wslabs = {}
        kcts, vtts = [], []
        for i, wname in enumerate(("wq8", "wk8", "wv8")):
            slab = pw.tile([128, 32, HPC * D], fp8, tag="slab" + wname)
            nc.sync.dma_start(out=slab[:], in_=w8[wname][:])
            wslabs[wname] = slab
            if i == 0:
                kct = pkc.tile([128, 2 * SQ], fp8, tag="kc")
                nc.sync.dma_start(out=kct[:], in_=kc8[0])
                kcts.append(kct)
            vtt = pvt.tile([128, NCH, VTW], fp8, tag="vt")
            nc.scalar.dma_start(out=vtt[:], in_=vt8[i])
            vtts.append(vtt)
        kct = pkc.tile([128, 2 * SQ], fp8, tag="kc")
        nc.sync.dma_start(out=kct[:], in_=kc8[1])
        kcts.append(kct)
        vpmn_sb = const.tile([8, NP, 128], bf16)
        nc.gpsimd.dma_start(out=vpmn_sb[:], in_=vpmn[:])
        kfull_sb = const.tile([128, NP, SFP], bf16)
        nc.gpsimd.dma_start(out=kfull_sb[:], in_=kfull[:])
        vfull_sb = const.tile([SFP, NP, 128], bf16)
        nc.gpsimd.dma_start(out=vfull_sb[:], in_=vfull[:])
        wo_sb = const.tile([128, HPC, HID], bf16)
        nc.gpsimd.dma_start(out=wo_sb[:], in_=woT[:])

        # ---- proj-q -> rope-q -> proj-k -> rope-k -> proj-v (PE-ordered) ----
        qscT = const.tile([128, NP], bf16)   # cols idx = h*B+b, scaled 1/sqrt(D)
        kT = const.tile([128, NP], bf16)
        proj_ps = {}

        def proj(wname):
            pps = psP.tile([B, HPC * D], f32, tag="proj" + wname)
            for kk in range(32):
                nc.tensor.matmul(pps[:], hid_sb[:, kk, :], wslabs[wname][:, kk, :],
                                 start=(kk == 0), stop=(kk == 31))
            proj_ps[wname] = pps

        def rope(wname, dst, scale):
            row_sb = const.tile([B, HPC * D], f32, tag="row" + wname)
            nc.scalar.copy(row_sb[:], proj_ps[wname][:])
            colT = const.tile([128, HPC, B], bf16, tag="colT" + wname)
            for h in range(HPC):
                pt = psR.tile([128, B], f32, tag="tmp")
                nc.tensor.transpose(pt[:], row_sb[0:B, h * D:(h + 1) * D], id4[:])
                nc.scalar.copy(colT[:, h, :], pt[:])
            dstv = dst[:].rearrange("p (h b) -> p h b", b=B)
            for b in range(B):
                ro = psR.tile([128, HPC], f32, tag="tmp")
                nc.tensor.matmul(ro[:], ropeM_sb[:, b, :], colT[:, :, b],
                                 start=True, stop=True)
                nc.scalar.mul(dstv[:, :, b], ro[:], scale)

        proj("wq8")
        rope("wq8", qscT, ISQD / WS)
        proj("wk8")
        rope("wk8", kT, 1.0 / WS)
        nc.gpsimd.dma_start(out=kfull_sb[:, :, SF:SFP], in_=kT[:])
        proj("wv8")
        v_sb = const.tile([B, HPC * D], bf16)
        nc.scalar.mul(v_sb[:], proj_ps["wv8"][:], 1.0 / WS)
        nc.gpsimd.dma_start(out=vfull_sb[SF:SFP, :, :], in_=v_sb[:])

        pctx.close()
        ictx = ctx.enter_context(ExitStack())
        psml = ictx.enter_context(tc.tile_pool(name="psml", bufs=3))
        psK = ictx.enter_context(tc.tile_pool(name="psK", bufs=2, space="PSUM"))
        psV = ictx.enter_context(tc.tile_pool(name="psV", bufs=2, space="PSUM"))
        psM = ictx.enter_context(tc.tile_pool(name="psM", bufs=4, space="PSUM"))

        woin_sb = const.tile([128, NP], bf16)
        state = {}

        def emit_K(idx):
            qcol = qscT[:, idx:idx + 1]
            if idx % 2 == 0 and idx >= 4:
                kct = pkc.tile([128, 2 * SQ], fp8, tag="kc")
                nc.sync.dma_start(out=kct[:], in_=kc8[idx // 2])
                kcts.append(kct)
            kct = kcts[idx // 2]
            if idx < 3:
                vtt = vtts[idx]
            else:
                vtt = pvt.tile([128, NCH, VTW], fp8, tag="vt")
                nc.scalar.dma_start(out=vtt[:], in_=vt8[idx])
            kcv = kct[:].rearrange("p (two s) -> p two s", two=2)[:, idx % 2, :]

            # qs8[d, g] = q_d * kscale*256 (fp8)
            qs8 = psml.tile([128, G], fp8, tag="qs8")
            nc.vector.tensor_tensor(qs8[:], blob_sb[:, idx, 0:64],
                                    qcol.to_broadcast((128, G)), AO.mult)
            psk = psK.tile([128, 2 * NCH], f32, tag="psk")
            for c in range(NCH):
                nc.tensor.matmul(psk[:, 2 * c:2 * c + 2], kcv[:, c * 128:(c + 1) * 128],
                                 qs8[:, 2 * c:2 * c + 2], start=True, stop=True)
            psm = psM.tile([128, 80], f32, tag="psm")
            nc.tensor.matmul(psm[0:SFP, 0:1], kfull_sb[:, idx, :], qcol,
                             start=True, stop=True)
            nc.tensor.matmul(psm[0:1, 4:72], qcol, blob_sb[:, idx, 64:132],
                             start=True, stop=True)
            qrmn_sb = psml.tile([1, 68], bf16, tag="qrmn")
            nc.vector.tensor_copy(qrmn_sb[:], psm[0:1, 4:72])
            qrb = psml.tile([128, 68], bf16, tag="qrb")
            nc.gpsimd.partition_broadcast(qrb[:], qrmn_sb[:])

            kpv = blob_sb[:, idx, 132:132 + NCH * RANK].rearrange(
                "p (c r) -> p c r", r=RANK)
            lrt = psml.tile([128, NCH, RANK], f32, tag="lrt")
            nc.vector.tensor_tensor(lrt[:], kpv,
                                    qrb[:, None, 0:4].to_broadcast((128, NCH, RANK)),
                                    AO.mult)
            lr = psml.tile([128, NCH], f32, tag="lr")
            nc.vector.reduce_sum(lr[:], lrt[:], axis=mybir.AxisListType.X)

            att = psml.tile([128, NCH + 1], f32, tag="att")
            pskv = psk[:].rearrange("p (c two) -> p c two", two=2)
            bbv = qrb[:, 4:68].rearrange("p (c two) -> p c two", two=2)
            nc.vector.scalar_tensor_tensor(att[0:64, 0:NCH], pskv[0:64, :, 0],
                                           1.0 / QS, lr[0:64, :], AO.mult, AO.add)
            nc.vector.scalar_tensor_tensor(att[64:128, 0:NCH], pskv[64:128, :, 1],
                                           1.0 / QS, lr[64:128, :], AO.mult, AO.add)
            nc.vector.tensor_tensor(att[0:64, 0:NCH], att[0:64, 0:NCH],
                                    bbv[0:64, :, 0], AO.add)
            nc.vector.tensor_tensor(att[64:128, 0:NCH], att[64:128, 0:NCH],
                                    bbv[64:128, :, 1], AO.add)
            nc.vector.tensor_copy(att[0:SFP, NCH:NCH + 1], psm[0:SFP, 0:1])
            nc.vector.memset(att[SFP:128, NCH:NCH + 1], -1e30)

            e = psml.tile([128, NCH + 1], bf16, tag="e")
            ssum = psml.tile([128, 1], f32, tag="ssum")
            nc.scalar.activation(e[:], att[:], AF.Exp, accum_out=ssum[:])
            sg = psml.tile([128, 1], f32, tag="sg")
            nc.gpsimd.partition_all_reduce(sg[:], ssum[:], 128, bass_isa.ReduceOp.add)
            state[idx] = (vtt, psm, e, sg)

        def emit_V(idx):
            vtt, psm, e, sg = state.pop(idx)
            vidx = (idx % B) * HPC + idx // B
            recip = psml.tile([128, 1], f32, tag="recip")
            nc.vector.reciprocal(recip[:], sg[:])
            recipV = psml.tile([128, 1], f32, tag="recipV")
            nc.vector.tensor_scalar(recipV[:], recip[:], 1.0 / VS, None, AO.mult)

            vscv = blob_sb[:, idx, 260:324].rearrange("p (c two) -> p c two", two=2)
            nc.vector.scalar_tensor_tensor(
                vtt[:, :, 144:146], e[:, 0:NCH, None].to_broadcast((128, NCH, 2)),
                VS, vscv, AO.mult, AO.mult)
            nc.vector.tensor_copy(vtt[:, :, 128:129], e[:, 0:NCH, None])

            psv = psV.tile([16, 132], f32, tag="psv")
            for c in range(NCH // 2):
                nc.tensor.matmul(psv[:], vtt[:, 2 * c:2 * c + 2, 144:160],
                                 vtt[:, 2 * c:2 * c + 2, 0:132],
                                 start=(c == 0), stop=(c == NCH // 2 - 1),
                                 perf_mode=PM.DoubleRow)

            awf = psml.tile([SFP, 1], bf16, tag="awf")
            nc.vector.tensor_scalar(awf[:], e[0:SFP, NCH:NCH + 1],
                                    recip[0:SFP, 0:1], None, AO.mult)
            nc.tensor.matmul(psm[:, 76:77], vfull_sb[:, vidx, :], awf[:],
                             start=True, stop=False)
            colsb = psml.tile([8, 1], bf16, tag="colsb")
            nc.vector.tensor_scalar(colsb[:], psv[0:8, 128:129],
                                    recip[0:8, 0:1], None, AO.mult)
            nc.tensor.matmul(psm[:, 76:77], vpmn_sb[:, idx, :], colsb[:],
                             start=False, stop=True)

            vT = psml.tile([2, 128], f32, tag="vT")
            nc.vector.tensor_copy(vT[:], psv[0:2, 0:128])
            nc.tensor.transpose(psm[:, 74:76], vT[:], id4[0:2, 0:2])
            wsb = psml.tile([128, 1], f32, tag="wsb")
            nc.vector.tensor_copy(wsb[:], psm[:, 76:77])
            nc.vector.scalar_tensor_tensor(woin_sb[0:64, idx:idx + 1],
                                           psm[0:64, 74:75], recipV[0:64, 0:1],
                                           wsb[0:64, :], AO.mult, AO.add)
            nc.vector.scalar_tensor_tensor(woin_sb[64:128, idx:idx + 1],
                                           psm[64:128, 75:76], recipV[64:128, 0:1],
                                           wsb[64:128, :], AO.mult, AO.add)

        # software pipeline: K(i) ahead of V(i-1)
        for i in range(NP + 1):
            if i < NP:
                emit_K(i)
            if i > 0:
                emit_V(i - 1)

        # ---- tail: wo matmul ----
        ictx.close()
        psO = ctx.enter_context(tc.tile_pool(name="psO", bufs=2, space="PSUM"))
        for half in range(2):
            po = psO.tile([B, HID // 2], f32, tag="po")
            for h in range(HPC):
                for nb in range(4):
                    j0 = half * 2048 + nb * 512
                    nc.tensor.matmul(po[:, nb * 512:(nb + 1) * 512],
                                     woin_sb[:, h * B:(h + 1) * B],
                                     wo_sb[:, h, j0:j0 + 512],
                                     start=(h == 0), stop=(h == HPC - 1))
            osb = const.tile([B, HID // 2], f32, tag=f"osb{half}")
            nc.scalar.copy(osb[:], po[:])
            nc.sync.dma_start(out=out[:, half * 2048:(half + 1) * 2048], in_=osb[:])

    nc.compile()
    return nc


def _host_prep(inputs):
    f8 = mybir.dt.np(mybir.dt.float8e4)
    bf = mybir.dt.np(mybir.dt.bfloat16)
    hs = np.asarray(inputs["hidden_states"], np.float32)
    pos = np.asarray(inputs["position_ids"])

    # rope matrices M_b^T (bf16): q_roped = M_b @ q
    inv = 1.0 / (THETA ** (np.arange(0, D, 2, dtype=np.float32) / D))
    fr = pos[:, 0].astype(np.float32)[:, None] * inv[None, :]
    emb = np.concatenate([fr, fr], axis=1)          # [B, 128]
    cos_b, sin_b = np.cos(emb), np.sin(emb)
    M = np.zeros((B, D, D), np.float32)
    dd = np.arange(D)
    M[:, dd, dd] = cos_b
    M[:, dd[:64], dd[:64] + 64] = -sin_b[:, :64]
    M[:, dd[64:], dd[64:] - 64] = sin_b[:, 64:]
    ropeM = np.ascontiguousarray(
        M.transpose(2, 0, 1)).astype(bf)             # [128(k), B, 128(m)]

    hidT = hs[:, 0, :].T                             # [HID, B]
    hid8 = np.ascontiguousarray(
        hidT.reshape(32, 128, B).transpose(1, 0, 2)).astype(f8)

    wq, wk, wv, wo = (np.asarray(inputs[k], np.float32) for k in ("wq", "wk", "wv", "wo"))
    kq_all = np.asarray(inputs["k_quant"], np.int32)
    ks_all = np.asarray(inputs["k_scale"], np.float32)
    km_all = np.asarray(inputs["k_mn"], np.float32)
    kf_all = np.asarray(inputs["k_full"], np.float32)
    kp_all = np.asarray(inputs["key_p"], np.float32)
    keyq_all = np.asarray(inputs["key_q"], np.float32)
    vq_all = np.asarray(inputs["v_quant"], np.int32)
    vs_all = np.asarray(inputs["v_scale"], np.float32)
    vm_all = np.asarray(inputs["v_mn"], np.float32)
    vf_all = np.asarray(inputs["v_full"], np.float32)
    vvq_all = np.asarray(inputs["value_q"], np.float32)
    vvp_all = np.asarray(inputs["value_p"], np.float32)

    in_maps = []
    for core in range(NCORES):
        h0 = core * HPC
        sl = slice(h0 * D, (h0 + HPC) * D)
        hsl = slice(h0, h0 + HPC)

        def wslab(w):   # [512, HID] -> [128, 32, 512] fp8 (x WS)
            wT = w[sl].T * WS
            return np.ascontiguousarray(
                wT.reshape(32, 128, HPC * D).transpose(1, 0, 2)).astype(f8)

        blobc = np.zeros((128, NP, BLOBW), np.float32)
        kfullc = np.zeros((128, NP, SFP), np.float32)
        vfullc = np.zeros((SFP, NP, 128), np.float32)
        vpmnc = np.zeros((8, NP, 128), np.float32)
        for h in range(HPC):
            for b in range(B):
                idx = h * B + b
                vidx = b * HPC + h
                blobc[:, idx, 0:64] = ks_all[b, h0 + h] * QS
                blobc[:, idx, 64:68] = keyq_all[b, h0 + h]
                blobc[:, idx, 68:132] = km_all[b, h0 + h]
                kp = kp_all[b, h0 + h].reshape(NCH, 128, RANK)
                blobc[:, idx, 132:132 + NCH * RANK] = \
                    kp.transpose(1, 0, 2).reshape(128, NCH * RANK)
                vs = vs_all[b, h0 + h].reshape(NCH, 128, FD)
                blobc[:, idx, 260:324] = vs.transpose(1, 0, 2).reshape(128, NCH * FD)
                kfullc[:, idx, 0:SF] = kf_all[b, h0 + h].T
                vfullc[0:SF, vidx, :] = vf_all[b, h0 + h]
                vpmnc[2:6, idx, :] = vvp_all[b, h0 + h].T
                vpmnc[6, idx, 0:64] = 1.0
                vpmnc[7, idx, 64:128] = 1.0

        # kc8 [NP//2, 128, 2*SQ]: pairs (2j, 2j+1) share one DMA
        kq = kq_all[:, hsl]                          # [B, HPC, 128, SQ]
        kcc = np.empty((NP // 2, 128, 2 * SQ), f8)
        for j in range(NP // 2):
            for t in range(2):
                idx = 2 * j + t
                h, b = idx // B, idx % B
                kcc[j, :, t * SQ:(t + 1) * SQ] = kq[b, h].astype(f8)

        # vt blob [NP, 128, NCH, VTW]: codes | e-slot | pad | aw3ext(16)
        vq = vq_all[:, hsl].reshape(B, HPC, NCH, 128, D)
        vvq = vvq_all[:, hsl].reshape(B, HPC, NCH, 128, RANK)
        vm = vm_all[:, hsl].reshape(B, HPC, NCH, 128, FD)
        vtc = np.zeros((NP, 128, NCH, VTW), f8)
        for h in range(HPC):
            for b in range(B):
                idx = h * B + b
                vtc[idx, :, :, 0:128] = vq[b, h].transpose(1, 0, 2).astype(f8)
                vtc[idx, :, :, 146:150] = vvq[b, h].transpose(1, 0, 2).astype(f8)
                vtc[idx, :, :, 150:152] = vm[b, h].transpose(1, 0, 2).astype(f8)

        m = {
            "hid8": hid8, "ropeM": ropeM,
            "wq8": wslab(wq), "wk8": wslab(wk), "wv8": wslab(wv),
            "woT": np.ascontiguousarray(
                wo[:, sl].T.reshape(HPC, 128, HID).transpose(1, 0, 2)).astype(bf),
            "blob": blobc.astype(bf),
            "vpmn": vpmnc.astype(bf),
            "kfull": kfullc.astype(bf),
            "vfull": vfullc.astype(bf),
            "kc8": kcc,
            "vt8": vtc,
        }
        in_maps.append(m)
    return in_maps


def kernel(**inputs):
    if "nc" not in _CACHE:
        _CACHE["nc"] = _build()
    nc = _CACHE["nc"]
    in_maps = _host_prep(inputs)
    res = run_bass_kernel_spmd(nc, in_maps, list(range(NCORES)),
                               trace=bool(os.environ.get("K_TRACE")))
    kernel.last = res
    total = np.zeros((B, HID), np.float32)
    for r in res.results:
        total += r["out"]
    return total.reshape(B, QL, HID)


# revision 21
# speedup vs baseline: 1.0204x; 1.0021x over previous
"""GEAR quantized-KV Llama attention decode step on 8 trn2 NeuronCores.

Sharding: tensor-parallel over heads (4 heads/core x 8 cores), all batches on
every core; each core computes a partial wo-product, summed on host.

v3: fp8 codes + fp8 matmuls (DoubleRow on V side), merged per-pair blob DMAs,
software-pipelined K/V phases, early wo load, DVE-staged copies.
"""
import os
import sys
import math

sys.path.insert(0, "/opt/trn_rl_repo")
import numpy as np
from contextlib import ExitStack

import concourse.bass as bass
import concourse.mybir as mybir
import concourse.tile as tile
from concourse import bacc, bass_isa
from concourse.bass_utils import run_bass_kernel_spmd
from concourse.masks import make_identity

B, H, D, HID = 4, 32, 128, 4096
SQ, SF, QL = 4096, 63, 1
GS, RANK = 64, 4
THETA = 10000.0
NCORES = 8
HPC = H // NCORES          # heads per core = 4
NP = B * HPC               # (b,h) pairs per core = 16
NCH = SQ // 128            # 32 s-chunks
G = SQ // GS               # 64 groups along seq (K side)
FD = D // GS               # 2 groups along head_dim (V side)
SFP = SF + 1               # 64 full-precision keys incl the new token
DT = mybir.dt
ISQD = 1.0 / math.sqrt(D)
WS = 16.0                  # fp8 weight pre-scale
QS = 256.0                 # fp8 qs pre-scale (folded into kscale on host)
VS = 16.0                  # fp8 aw*vscale pre-scale
VTW = 160                  # vt row: 128 codes | e | pad | aw3ext@144 (16B-aligned)
BLOBW = 64 + 68 + NCH * RANK + NCH * FD  # 324

_CACHE = {}


def _build():
    nc = bacc.Bacc("TRN2", target_bir_lowering=False)
    f32, bf16, fp8 = DT.float32, DT.bfloat16, DT.float8e4

    # ---- DRAM parameters (per core) ----
    hid8 = nc.declare_dram_parameter("hid8", [128, 32, B], fp8, isOutput=False)
    w8 = {w: nc.declare_dram_parameter(w, [128, 32, HPC * D], fp8, isOutput=False)
          for w in ("wq8", "wk8", "wv8")}
    woT = nc.declare_dram_parameter("woT", [128, HPC, HID], bf16, isOutput=False)
    ropeM = nc.declare_dram_parameter("ropeM", [128, B, 128], bf16, isOutput=False)
    blob = nc.declare_dram_parameter("blob", [128, NP, BLOBW], bf16, isOutput=False)
    vpmn = nc.declare_dram_parameter("vpmn", [8, NP, 128], bf16, isOutput=False)
    kfull = nc.declare_dram_parameter("kfull", [128, NP, SFP], bf16, isOutput=False)
    vfull = nc.declare_dram_parameter("vfull", [SFP, NP, 128], bf16, isOutput=False)
    kc8 = nc.declare_dram_parameter("kc8", [NP // 2, 128, 2 * SQ], fp8, isOutput=False)
    vt8 = nc.declare_dram_parameter("vt8", [NP, 128, NCH, VTW], fp8, isOutput=False)
    out = nc.declare_dram_parameter("out", [B, HID], f32, isOutput=True)

    AO = mybir.AluOpType
    AF = mybir.ActivationFunctionType
    PM = mybir.MatmulPerfMode

    with tile.TileContext(nc) as tc, ExitStack() as ctx:
        const = ctx.enter_context(tc.tile_pool(name="const", bufs=1))
        pkc = ctx.enter_context(tc.tile_pool(name="pkc", bufs=2))
        pvt = ctx.enter_context(tc.tile_pool(name="pvt", bufs=16))
        psml = ctx.enter_context(tc.tile_pool(name="psml", bufs=3))
        pctx2 = ctx.enter_context(ExitStack())
        pw = pctx2.enter_context(tc.tile_pool(name="pw", bufs=2))
        psP = pctx2.enter_context(tc.tile_pool(name="psP", bufs=1, space="PSUM"))
        pctx1 = ctx.enter_context(ExitStack())
        psR = pctx1.enter_context(tc.tile_pool(name="psR", bufs=2, space="PSUM"))

        # ---- constants ----
        id4 = const.tile([4, 4], f32)
        make_identity(nc, id4[:])

        # ---- upfront loads ----
        hid_sb = const.tile([128, 32, B], fp8)
        nc.sync.dma_start(out=hid_sb[:], in_=hid8[:])
        ropeM_sb = const.tile([128, B, 128], bf16)
        nc.sync.dma_start(out=ropeM_sb[:], in_=ropeM[:])
        blob_sb = const.tile([128, NP, BLOBW], bf16)
        nc.scalar.dma_start(out=blob_sb[:], in_=blob[:])
        wslabs = {}
        kcts, vtts = [], []
        for i, wname in enumerate(("wq8", "wk8")):
            slab = pw.tile([128, 32, HPC * D], fp8, tag="slab")
            nc.sync.dma_start(out=slab[:], in_=w8[wname][:])
            wslabs[wname] = slab
            vtt = pvt.tile([128, NCH, VTW], fp8, tag="vt")
            nc.scalar.dma_start(out=vtt[:], in_=vt8[i])
            vtts.append(vtt)
        vtt = pvt.tile([128, NCH, VTW], fp8, tag="vt")
        nc.scalar.dma_start(out=vtt[:], in_=vt8[2])
        vtts.append(vtt)
        for j in range(2):
            kct = pkc.tile([128, 2 * SQ], fp8, tag="kc")
            nc.sync.dma_start(out=kct[:], in_=kc8[j])
            kcts.append(kct)
        slab = pw.tile([128, 32, HPC * D], fp8, tag="slab")
        nc.sync.dma_start(out=slab[:], in_=w8["wv8"][:])
        wslabs["wv8"] = slab
        vpmn_sb = const.tile([8, NP, 128], bf16)
        nc.gpsimd.dma_start(out=vpmn_sb[:], in_=vpmn[:])
        kfull_sb = const.tile([128, NP, SFP], bf16)
        nc.gpsimd.dma_start(out=kfull_sb[:], in_=kfull[:])
        vfull_sb = const.tile([SFP, NP, 128], bf16)
        nc.gpsimd.dma_start(out=vfull_sb[:], in_=vfull[:])
        wo_sb = const.tile([128, HPC, HID], bf16)
        nc.gpsimd.dma_start(out=wo_sb[:], in_=woT[:])

        # ---- PE warm-up: keep tensor engine busy while weights stream in,
        #      so DVFS ramps to full clock before the projections ----
        hidv = hid_sb[:].rearrange("p c b -> p (c b)")
        for w in range(110):
            wu = psR.tile([B, 128], f32, tag="wu")
            nc.tensor.matmul(wu[:], hidv[:, 0:B], hidv[:], start=True, stop=True)

        # ---- proj-q -> rope-q -> proj-k -> rope-k (PE-ordered) ----
        qscT = const.tile([128, NP], bf16)   # cols idx = h*B+b, scaled 1/sqrt(D)
        kT = const.tile([128, NP], bf16)
        proj_ps = {w: psP.tile([B, HPC * D], f32, tag="proj" + w, name="pps_" + w)
                   for w in ("wq8", "wk8", "wv8")}

        def proj(wname):
            pps = proj_ps[wname]
            for kk in range(32):
                nc.tensor.matmul(pps[:], hid_sb[:, kk, :], wslabs[wname][:, kk, :],
                                 start=(kk == 0), stop=(kk == 31))

        def rope(wname, dst, scale):
            row_sb = const.tile([B, HPC * D], f32, tag="row" + wname)
            nc.scalar.copy(row_sb[:], proj_ps[wname][:])
            colT = const.tile([128, HPC, B], bf16, tag="colT" + wname)
            for h in range(HPC):
                pt = psR.tile([128, B], f32, tag="tmp")
                nc.tensor.transpose(pt[:], row_sb[0:B, h * D:(h + 1) * D], id4[:])
                nc.scalar.copy(colT[:, h, :], pt[:])
            dstv = dst[:].rearrange("p (h b) -> p h b", b=B)
            for b in range(B):
                ro = psR.tile([128, HPC], f32, tag="tmp")
                nc.tensor.matmul(ro[:], ropeM_sb[:, b, :], colT[:, :, b],
                                 start=True, stop=True)
                nc.scalar.mul(dstv[:, :, b], ro[:], scale)

        proj("wq8")
        rope("wq8", qscT, ISQD / WS)
        proj("wk8")
        rope("wk8", kT, 1.0 / WS)
        nc.gpsimd.dma_start(out=kfull_sb[:, :, SF:SFP], in_=kT[:])
        pctx1.close()

        kctx = ctx.enter_context(ExitStack())
        psK = kctx.enter_context(tc.tile_pool(name="psK", bufs=2, space="PSUM"))
        psMK = kctx.enter_context(tc.tile_pool(name="psMK", bufs=2, space="PSUM"))

        woin_sb = const.tile([128, NP], bf16)
        e_all = const.tile([128, NP, NCH + 1], bf16)
        sg_all = const.tile([128, NP], f32)

        # ================= K phase =================
        def emit_K(idx):
            qcol = qscT[:, idx:idx + 1]
            if idx % 2 == 0 and idx >= 4:
                kct = pkc.tile([128, 2 * SQ], fp8, tag="kc")
                nc.sync.dma_start(out=kct[:], in_=kc8[idx // 2])
                kcts.append(kct)
            kct = kcts[idx // 2]
            if idx >= 3:
                vtt = pvt.tile([128, NCH, VTW], fp8, tag="vt")
                nc.scalar.dma_start(out=vtt[:], in_=vt8[idx])
                vtts.append(vtt)
            kcv = kct[:].rearrange("p (two s) -> p two s", two=2)[:, idx % 2, :]

            qs8 = psml.tile([128, G], fp8, tag="qs8")
            nc.vector.tensor_tensor(qs8[:], blob_sb[:, idx, 0:64],
                                    qcol.to_broadcast((128, G)), AO.mult)
            psk = psK.tile([128, 2 * NCH], f32, tag="psk")
            for c in range(NCH):
                nc.tensor.matmul(psk[:, 2 * c:2 * c + 2], kcv[:, c * 128:(c + 1) * 128],
                                 qs8[:, 2 * c:2 * c + 2], start=True, stop=True)
            psm = psMK.tile([128, 80], f32, tag="psm")
            nc.tensor.matmul(psm[0:SFP, 0:1], kfull_sb[:, idx, :], qcol,
                             start=True, stop=True)
            nc.tensor.matmul(psm[0:1, 4:72], qcol, blob_sb[:, idx, 64:132],
                             start=True, stop=True)
            qrmn_sb = psml.tile([1, 68], bf16, tag="qrmn")
            nc.vector.tensor_copy(qrmn_sb[:], psm[0:1, 4:72])
            qrb = psml.tile([128, 68], bf16, tag="qrb")
            nc.gpsimd.partition_broadcast(qrb[:], qrmn_sb[:])

            kpv = blob_sb[:, idx, 132:132 + NCH * RANK].rearrange(
                "p (c r) -> p c r", r=RANK)
            lrt = psml.tile([128, NCH, RANK], f32, tag="lrt")
            nc.vector.tensor_tensor(lrt[:], kpv,
                                    qrb[:, None, 0:4].to_broadcast((128, NCH, RANK)),
                                    AO.mult)
            lr = psml.tile([128, NCH], f32, tag="lr")
            nc.vector.reduce_sum(lr[:], lrt[:], axis=mybir.AxisListType.X)

            att = psml.tile([128, NCH + 1], f32, tag="att")
            pskv = psk[:].rearrange("p (c two) -> p c two", two=2)
            bbv = qrb[:, 4:68].rearrange("p (c two) -> p c two", two=2)
            nc.vector.scalar_tensor_tensor(att[0:64, 0:NCH], pskv[0:64, :, 0],
                                           1.0 / QS, lr[0:64, :], AO.mult, AO.add)
            nc.vector.scalar_tensor_tensor(att[64:128, 0:NCH], pskv[64:128, :, 1],
                                           1.0 / QS, lr[64:128, :], AO.mult, AO.add)
            nc.vector.tensor_tensor(att[0:64, 0:NCH], att[0:64, 0:NCH],
                                    bbv[0:64, :, 0], AO.add)
            nc.vector.tensor_tensor(att[64:128, 0:NCH], att[64:128, 0:NCH],
                                    bbv[64:128, :, 1], AO.add)
            nc.vector.tensor_copy(att[0:SFP, NCH:NCH + 1], psm[0:SFP, 0:1])
            nc.vector.memset(att[SFP:128, NCH:NCH + 1], -1e30)

            ssum = psml.tile([128, 1], f32, tag="ssum")
            nc.scalar.activation(e_all[:, idx, :], att[:], AF.Exp, accum_out=ssum[:])
            nc.gpsimd.partition_all_reduce(sg_all[:, idx:idx + 1], ssum[:], 128,
                                           bass_isa.ReduceOp.add)

        for i in range(NP):
            emit_K(i)

        # v projection + new-token value row (needed from V phase on)
        kctx.close()
        proj("wv8")
        v_sb = const.tile([B, HPC * D], bf16)
        nc.scalar.mul(v_sb[:], proj_ps["wv8"][:], 1.0 / WS)
        nc.sync.dma_start(out=vfull_sb[SF:SFP, :, :], in_=v_sb[:])
        pctx2.close()

        ictx = ctx.enter_context(ExitStack())
        psV = ictx.enter_context(tc.tile_pool(name="psV", bufs=2, space="PSUM"))
        psMV = ictx.enter_context(tc.tile_pool(name="psMV", bufs=2, space="PSUM"))

        # ================= V phase =================
        def emit_V(idx):
            vtt = vtts[idx]
            e = e_all[:, idx, :]
            vidx = (idx % B) * HPC + idx // B
            recip = psml.tile([128, 1], f32, tag="recip")
            nc.vector.reciprocal(recip[:], sg_all[:, idx:idx + 1])
            recipV = psml.tile([128, 1], f32, tag="recipV")
            nc.vector.tensor_scalar(recipV[:], recip[:], 1.0 / VS, None, AO.mult)

            vscv = blob_sb[:, idx, 260:324].rearrange("p (c two) -> p c two", two=2)
            nc.vector.scalar_tensor_tensor(
                vtt[:, :, 144:146], e[:, 0:NCH, None].to_broadcast((128, NCH, 2)),
                VS, vscv, AO.mult, AO.mult)
            nc.vector.tensor_copy(vtt[:, :, 128:129], e[:, 0:NCH, None])

            psv = psV.tile([16, 132], f32, tag="psv")
            for c in range(NCH // 2):
                nc.tensor.matmul(psv[:], vtt[:, 2 * c:2 * c + 2, 144:160],
                                 vtt[:, 2 * c:2 * c + 2, 0:132],
                                 start=(c == 0), stop=(c == NCH // 2 - 1),
                                 perf_mode=PM.DoubleRow)

            psm = psMV.tile([128, 4], f32, tag="psmv")
            awf = psml.tile([SFP, 1], bf16, tag="awf")
            nc.scalar.mul(awf[:], e[0:SFP, NCH:NCH + 1], recip[0:SFP, 0:1])
            nc.tensor.matmul(psm[:, 2:3], vfull_sb[:, vidx, :], awf[:],
                             start=True, stop=False)
            colsb = psml.tile([8, 1], bf16, tag="colsb")
            nc.scalar.mul(colsb[:], psv[0:8, 128:129], recip[0:8, 0:1])
            nc.tensor.matmul(psm[:, 2:3], vpmn_sb[:, idx, :], colsb[:],
                             start=False, stop=True)

            vT = psml.tile([2, 128], f32, tag="vT")
            nc.scalar.copy(vT[:], psv[0:2, 0:128])
            nc.tensor.transpose(psm[:, 0:2], vT[:], id4[0:2, 0:2])
            wsb = psml.tile([128, 1], f32, tag="wsb")
            nc.scalar.copy(wsb[:], psm[:, 2:3])
            nc.vector.scalar_tensor_tensor(woin_sb[0:64, idx:idx + 1],
                                           psm[0:64, 0:1], recipV[0:64, 0:1],
                                           wsb[0:64, :], AO.mult, AO.add)
            nc.vector.scalar_tensor_tensor(woin_sb[64:128, idx:idx + 1],
                                           psm[64:128, 1:2], recipV[64:128, 0:1],
                                           wsb[64:128, :], AO.mult, AO.add)

        for i in range(NP):
            emit_V(i)

        # ---- tail: wo matmul ----
        ictx.close()
        psO = ctx.enter_context(tc.tile_pool(name="psO", bufs=2, space="PSUM"))
        for half in range(2):
            po = psO.tile([B, HID // 2], f32, tag="po")
            for h in range(HPC):
                for nb in range(4):
                    j0 = half * 2048 + nb * 512
                    nc.tensor.matmul(po[:, nb * 512:(nb + 1) * 512],
                                     woin_sb[:, h * B:(h + 1) * B],
                                     wo_sb[:, h, j0:j0 + 512],
                                     start=(h == 0), stop=(h == HPC - 1))
            osb = const.tile([B, HID // 2], f32, tag="osb")
            nc.scalar.copy(osb[:], po[:])
            nc.sync.dma_start(out=out[:, half * 2048:(half + 1) * 2048], in_=osb[:])

    nc.compile()
    return nc


def _host_prep(inputs):
    f8 = mybir.dt.np(mybir.dt.float8e4)
    bf = mybir.dt.np(mybir.dt.bfloat16)
    hs = np.asarray(inputs["hidden_states"], np.float32)
    pos = np.asarray(inputs["position_ids"])

    # rope matrices M_b^T (bf16): q_roped = M_b @ q
    inv = 1.0 / (THETA ** (np.arange(0, D, 2, dtype=np.float32) / D))
    fr = pos[:, 0].astype(np.float32)[:, None] * inv[None, :]
    emb = np.concatenate([fr, fr], axis=1)          # [B, 128]
    cos_b, sin_b = np.cos(emb), np.sin(emb)
    M = np.zeros((B, D, D), np.float32)
    dd = np.arange(D)
    M[:, dd, dd] = cos_b
    M[:, dd[:64], dd[:64] + 64] = -sin_b[:, :64]
    M[:, dd[64:], dd[64:] - 64] = sin_b[:, 64:]
    ropeM = np.ascontiguousarray(
        M.transpose(2, 0, 1)).astype(bf)             # [128(k), B, 128(m)]

    hidT = hs[:, 0, :].T                             # [HID, B]
    hid8 = np.ascontiguousarray(
        hidT.reshape(32, 128, B).transpose(1, 0, 2)).astype(f8)

    wq, wk, wv, wo = (np.asarray(inputs[k], np.float32) for k in ("wq", "wk", "wv", "wo"))
    kq_all = np.asarray(inputs["k_quant"], np.int32)
    ks_all = np.asarray(inputs["k_scale"], np.float32)
    km_all = np.asarray(inputs["k_mn"], np.float32)
    kf_all = np.asarray(inputs["k_full"], np.float32)
    kp_all = np.asarray(inputs["key_p"], np.float32)
    keyq_all = np.asarray(inputs["key_q"], np.float32)
    vq_all = np.asarray(inputs["v_quant"], np.int32)
    vs_all = np.asarray(inputs["v_scale"], np.float32)
    vm_all = np.asarray(inputs["v_mn"], np.float32)
    vf_all = np.asarray(inputs["v_full"], np.float32)
    vvq_all = np.asarray(inputs["value_q"], np.float32)
    vvp_all = np.asarray(inputs["value_p"], np.float32)

    in_maps = []
    for core in range(NCORES):
        h0 = core * HPC
        sl = slice(h0 * D, (h0 + HPC) * D)
        hsl = slice(h0, h0 + HPC)

        def wslab(w):   # [512, HID] -> [128, 32, 512] fp8 (x WS)
            wT = w[sl].T * WS
            return np.ascontiguousarray(
                wT.reshape(32, 128, HPC * D).transpose(1, 0, 2)).astype(f8)

        blobc = np.zeros((128, NP, BLOBW), np.float32)
        kfullc = np.zeros((128, NP, SFP), np.float32)
        vfullc = np.zeros((SFP, NP, 128), np.float32)
        vpmnc = np.zeros((8, NP, 128), np.float32)
        for h in range(HPC):
            for b in range(B):
                idx = h * B + b
                vidx = b * HPC + h
                blobc[:, idx, 0:64] = ks_all[b, h0 + h] * QS
                blobc[:, idx, 64:68] = keyq_all[b, h0 + h]
                blobc[:, idx, 68:132] = km_all[b, h0 + h]
                kp = kp_all[b, h0 + h].reshape(NCH, 128, RANK)
                blobc[:, idx, 132:132 + NCH * RANK] = \
                    kp.transpose(1, 0, 2).reshape(128, NCH * RANK)
                vs = vs_all[b, h0 + h].reshape(NCH, 128, FD)
                blobc[:, idx, 260:324] = vs.transpose(1, 0, 2).reshape(128, NCH * FD)
                kfullc[:, idx, 0:SF] = kf_all[b, h0 + h].T
                vfullc[0:SF, vidx, :] = vf_all[b, h0 + h]
                vpmnc[2:6, idx, :] = vvp_all[b, h0 + h].T
                vpmnc[6, idx, 0:64] = 1.0
                vpmnc[7, idx, 64:128] = 1.0

        # kc8 [NP//2, 128, 2*SQ]: pairs (2j, 2j+1) share one DMA
        kq = kq_all[:, hsl]                          # [B, HPC, 128, SQ]
        kcc = np.empty((NP // 2, 128, 2 * SQ), f8)
        for j in range(NP // 2):
            for t in range(2):
                idx = 2 * j + t
                h, b = idx // B, idx % B
                kcc[j, :, t * SQ:(t + 1) * SQ] = kq[b, h].astype(f8)

        # vt blob [NP, 128, NCH, VTW]: codes | e-slot | pad | aw3ext(16)
        vq = vq_all[:, hsl].reshape(B, HPC, NCH, 128, D)
        vvq = vvq_all[:, hsl].reshape(B, HPC, NCH, 128, RANK)
        vm = vm_all[:, hsl].reshape(B, HPC, NCH, 128, FD)
        vtc = np.zeros((NP, 128, NCH, VTW), f8)
        for h in range(HPC):
            for b in range(B):
                idx = h * B + b
                vtc[idx, :, :, 0:128] = vq[b, h].transpose(1, 0, 2).astype(f8)
                vtc[idx, :, :, 146:150] = vvq[b, h].transpose(1, 0, 2).astype(f8)
                vtc[idx, :, :, 150:152] = vm[b, h].transpose(1, 0, 2).astype(f8)

        m = {
            "hid8": hid8, "ropeM": ropeM,
            "wq8": wslab(wq), "wk8": wslab(wk), "wv8": wslab(wv),
            "woT": np.ascontiguousarray(
                wo[:, sl].T.reshape(HPC, 128, HID).transpose(1, 0, 2)).astype(bf),
            "blob": blobc.astype(bf),
            "vpmn": vpmnc.astype(bf),
            "kfull": kfullc.astype(bf),
            "vfull": vfullc.astype(bf),
            "kc8": kcc,
            "vt8": vtc,
        }
        in_maps.append(m)
    return in_maps


def kernel(**inputs):
    if "nc" not in _CACHE:
        _CACHE["nc"] = _build()
    nc = _CACHE["nc"]
    in_maps = _host_prep(inputs)
    res = run_bass_kernel_spmd(nc, in_maps, list(range(NCORES)),
                               trace=bool(os.environ.get("K_TRACE")))
    kernel.last = res
    total = np.zeros((B, HID), np.float32)
    for r in res.results:
        total += r["out"]
    return total.reshape(B, QL, HID)
